# revision 24
# baseline (speedup 1.0000x reference)
"""Trainium2 Bass kernel for CompositionalFC (moe_routing).

Reference computation:
    z[n,b,o] = x[b,i] @ weight[n,i,o] + bias[n,o]
    out[b,o] = relu( sum_n comp_weight[b,n] * z[n,b,o] )

Strategy: data-parallel over batch across 8 NeuronCores (512 rows
each), expert matmuls in fp8e4 DoubleRow mode (216 ns per 512-col pass
= fp8 PE roofline, ~157 TF/s/core). 512 main passes = 110.6 us of PE
work and the ENTIRE combine is folded into PSUM accumulation:

  - the host ships per-expert pre-scaled activations
    xs[n] = fp8((x - is folded) .. = fp8(x * c[:,n]) in the lhsT layout,
    so every expert's passes accumulate c_n * (x @ v_n) directly into
    EIGHT RESIDENT PSUM BANKS (one per (batch-tile, output-half) pair =
    all of PSUM). There are NO Vector combines at all (the v7 design
    spent ~83us of Vector time on them and its tail slid ~1.5us behind
    queue congestion). Same fp8 relative error as quantizing x alone
    (sim: l2 7.3e-3 vs 2e-2 gate).
  - w = 0.5 + v, v quantized to fp8e4; the exact rank-1 term
    0.5*rowsum(x)*sum_c is computed ON HOST (f64) and folded into the
    final drain's per-partition f32 bias r1.
  - each resident group OPENS with a K=16 bf16 seed pass cT@bias (the
    c @ bias term), gated only on the 48KB cb DMA during the ~10-pass
    junk warm-up window (every DMA gate costs data time + ~2.5us
    sem-visibility lag, so the seed must not sit on the weight path).
  - every expert runs kt-phased (kt-outer over all 4 batch tiles, 8
    passes per kt round = 1.73us), which matches the ~1.7us arrival
    cadence of the interleaved xs/w quarter-DMAs at startup; weights
    and xs prefetch 4 experts ahead. Total DMA is 25.2MB/core
    (~228 GB/s sustained vs ~300 effective capacity).
  - the drain is just relu(psum + r1): expert 15's FINAL kt round runs
    batch-tile-major with the two per-tile drains (Vector takes ot0,
    Scalar takes ot1 - both can read PSUM) and the store issued inline,
    so stores spread across the last round and only the last tile's
    ot1 chain (~1.9us: ReLU -> issue -> flight) trails the final pass.
"""

import sys

for _p in ("/opt/trn_rl_repo",):
    if _p not in sys.path:
        sys.path.insert(0, _p)

from contextlib import ExitStack

import ml_dtypes
import numpy as np

import concourse.bass as bass
import concourse.mybir as mybir
import concourse.tile as tile
from concourse import bacc
from concourse.bass_utils import run_bass_kernel_spmd

N_CORES = 8
BATCH, IN_DIM, OUT_DIM, N_EXP = 4096, 1024, 1024, 16
BS = BATCH // N_CORES          # 512 batch rows per core
P = 128                        # partitions
BT = BS // P                   # 4 batch tiles per core
KT2 = IN_DIM // 256            # 4 DoubleRow contraction tiles (K=256 each)
FD = 512                       # matmul free dim / PSUM bank width (fp32)
NO = OUT_DIM // FD             # 2 output column tiles
N_JUNK = 10                    # PE p-state warm-up passes

F32 = mybir.dt.float32
BF16 = mybir.dt.bfloat16
F8 = mybir.dt.float8e4
DR = mybir.MatmulPerfMode.DoubleRow
ACT = mybir.ActivationFunctionType

E4NP = ml_dtypes.float8_e4m3   # TRN fp8e4 == IEEE e4m3 (max 240)


def _build_kernel():
    nc = bacc.Bacc(
        "TRN2",
        target_bir_lowering=False,
        debug=False,
        num_devices=N_CORES,
    )
    # k = kt2*256 + slot*128 + p; b = bt*128 + p_out
    xs8 = nc.declare_dram_parameter(
        "xs8", [N_EXP, P, KT2, 2, BS], F8, isOutput=False
    )
    w8 = nc.declare_dram_parameter("w8", [N_EXP, P, KT2, 2, OUT_DIM], F8, isOutput=False)
    cb = nc.declare_dram_parameter("cb", [N_EXP, BS + OUT_DIM], BF16, isOutput=False)
    r1 = nc.declare_dram_parameter("r1", [P, BT], F32, isOutput=False)
    out = nc.declare_dram_parameter("out", [P, BT, OUT_DIM], BF16, isOutput=True)

    with ExitStack() as ctx:
        tc = ctx.enter_context(tile.TileContext(nc))
        const = ctx.enter_context(tc.tile_pool(name="const", bufs=1))
        wpool = ctx.enter_context(tc.tile_pool(name="wpool", bufs=5))
        xpool = ctx.enter_context(tc.tile_pool(name="xpool", bufs=5))
        rpool = ctx.enter_context(tc.tile_pool(name="rpool", bufs=1, space="PSUM"))

        # --- junk-matmul operands (parallel memsets on two engines) -----
        ones8 = const.tile([P, 2, 16], F8, tag="ones8")
        nc.gpsimd.memset(ones8[:], 0.0)
        junk8 = const.tile([P, 2, FD], F8, tag="junk8")
        nc.vector.memset(junk8[:], 0.0)

        # --- startup DMAs on sync, most-gating first --------------------
        cb_sb = const.tile([N_EXP, BS + OUT_DIM], BF16, tag="cb_sb")
        nc.sync.dma_start(cb_sb[:], cb[:, :])

        w_sb = {}
        xs_sb = {}

        def alloc_wx(n):
            w_sb[n] = wpool.tile([P, KT2, 2, OUT_DIM], F8, name=f"w_{n}", tag="w_sb")
            xs_sb[n] = xpool.tile([P, KT2, 2, BS], F8, name=f"xs_{n}", tag="xs_sb")

        def fetch_wx(n, chunks=1):
            kq = KT2 // chunks
            for i in range(chunks):
                nc.sync.dma_start(
                    xs_sb[n][:, i * kq : (i + 1) * kq],
                    xs8[n, :, :][:, i * kq : (i + 1) * kq],
                )
                nc.sync.dma_start(
                    w_sb[n][:, i * kq : (i + 1) * kq],
                    w8[n, :, :][:, i * kq : (i + 1) * kq],
                )

        alloc_wx(0)
        fetch_wx(0, chunks=4)
        r1_sb = const.tile([P, BT], F32, tag="r1_sb")
        nc.sync.dma_start(r1_sb[:], r1[:, :])
        alloc_wx(1)
        fetch_wx(1, chunks=4)
        for n in (2, 3):
            alloc_wx(n)
            fetch_wx(n, chunks=2)

        ob_sb = [
            const.tile([P, NO, FD], BF16, name=f"ob_{bt}", tag=f"ob_{bt}")
            for bt in range(BT)
        ]

        # --- resident PSUM accumulators: one bank per (bt, ot) ----------
        res = [
            [
                rpool.tile(
                    [P, FD], F32, name=f"res_{bt}_{ot}", tag=f"res_{bt}_{ot}"
                )
                for ot in range(NO)
            ]
            for bt in range(BT)
        ]

        # --- PE clock warm-up: junk DR passes (write a resident bank
        # pre-seed; WAW on the same engine is ordered) -------------------
        for _ in range(N_JUNK):
            nc.tensor.matmul(
                res[BT - 1][NO - 1][0:1, :],
                lhsT=ones8[:, :, 0:1],
                rhs=junk8[:],
                start=True,
                stop=True,
                perf_mode=DR,
            )

        # --- open every group with its K=16 bf16 seed pass cT@bias ------
        for bt in range(BT):
            for ot in range(NO):
                nc.tensor.matmul(
                    res[bt][ot][:, :],
                    lhsT=cb_sb[:, bt * P : (bt + 1) * P],
                    rhs=cb_sb[:, BS + ot * FD : BS + (ot + 1) * FD],
                    start=True,
                    stop=False,
                )

        def drain(bt, ot):
            # relu(psum + r1) -> bf16; Vector takes ot0, Scalar ot1
            # (both engines can read PSUM; they run concurrently)
            if ot == 0:
                nc.vector.tensor_scalar(
                    out=ob_sb[bt][:, ot],
                    in0=res[bt][ot][:, :],
                    scalar1=r1_sb[:, bt : bt + 1],
                    scalar2=0.0,
                    op0=mybir.AluOpType.add,
                    op1=mybir.AluOpType.max,
                )
            else:
                nc.scalar.activation(
                    ob_sb[bt][:, ot],
                    res[bt][ot][:, :],
                    ACT.Relu,
                    bias=r1_sb[:, bt : bt + 1],
                )

        out_ap = out[:, :]

        # --- all experts: kt-phased (kt-outer over all 4 bt), passes
        # accumulate into the resident banks; expert 15's final kt round
        # drains + stores each tile inline ------------------------------
        for n in range(N_EXP):
            last_exp = n == N_EXP - 1
            for kt in range(KT2):
                last_round = last_exp and kt == KT2 - 1
                for bt in range(BT):
                    for ot in range(NO):
                        nc.tensor.matmul(
                            res[bt][ot][:, :],
                            lhsT=xs_sb[n][:, kt, :, bt * P : (bt + 1) * P],
                            rhs=w_sb[n][:, kt, :, ot * FD : (ot + 1) * FD],
                            start=False,
                            stop=last_round,
                            perf_mode=DR,
                        )
                    if last_round:
                        drain(bt, 0)
                        drain(bt, 1)
                        if bt < BT - 1:
                            nc.sync.dma_start(out_ap[:, bt, :], ob_sb[bt][:])
                        else:
                            # split the very last tile's store so the
                            # trailing chain moves only 128KB
                            nc.sync.dma_start(
                                out_ap[:, bt, 0:FD], ob_sb[bt][:, 0]
                            )
                            nc.sync.dma_start(
                                out_ap[:, bt, FD:OUT_DIM], ob_sb[bt][:, 1]
                            )
            if n + 4 < N_EXP:
                alloc_wx(n + 4)
                fetch_wx(n + 4, chunks=2)

    nc.compile()
    return nc


_NC_CACHE = {}


def _get_nc():
    if "nc" not in _NC_CACHE:
        _NC_CACHE["nc"] = _build_kernel()
    return _NC_CACHE["nc"]


def _xt_layout(x8):
    # fp8 [BS, IN_DIM] -> lhsT [P, KT2, 2, BS] with k = kt2*256+slot*128+p
    xT = np.ascontiguousarray(x8.T)  # [IN_DIM, BS]
    return np.ascontiguousarray(xT.reshape(KT2, 2, P, BS).transpose(2, 0, 1, 3))


def prepare_inputs(x, comp_weight, weight, bias):
    x = np.ascontiguousarray(np.asarray(x, dtype=np.float32))
    comp_weight = np.ascontiguousarray(np.asarray(comp_weight, dtype=np.float32))
    weight = np.asarray(weight, dtype=np.float32)
    bias = np.ascontiguousarray(np.asarray(bias, dtype=np.float32))

    # w = 0.5 + v; ship v in fp8 laid out [n, p, kt2, slot, o]
    v8 = (weight - np.float32(0.5)).astype(E4NP)
    w8 = np.ascontiguousarray(
        v8.reshape(N_EXP, KT2, 2, P, OUT_DIM).transpose(0, 3, 1, 2, 4)
    )
    bias_bf = bias.astype(ml_dtypes.bfloat16)

    # exact rank-1 ReLU bias: r1 = 0.5 * rowsum(x) * sum_c (f64 -> f32)
    r1_full = 0.5 * x.astype(np.float64).sum(1) * comp_weight.astype(np.float64).sum(1)

    in_maps = []
    for r in range(N_CORES):
        sl = slice(r * BS, (r + 1) * BS)
        xs = x[sl]
        cs = comp_weight[sl]
        # per-expert pre-scaled activations xs[n] = fp8(x * c[:, n])
        xs8 = np.stack(
            [_xt_layout((xs * cs[:, n : n + 1]).astype(E4NP)) for n in range(N_EXP)]
        )
        cb = np.concatenate(
            [cs.T.astype(ml_dtypes.bfloat16), bias_bf], axis=1
        )
        in_maps.append(
            {
                "xs8": np.ascontiguousarray(xs8),
                "w8": w8,
                "cb": np.ascontiguousarray(cb),
                "r1": np.ascontiguousarray(
                    r1_full[sl].astype(np.float32).reshape(BT, P).T
                ),
            }
        )
    return in_maps


def _run(x, comp_weight, weight, bias, trace=False):
    in_maps = prepare_inputs(x, comp_weight, weight, bias)
    res = run_bass_kernel_spmd(
        _get_nc(), in_maps, core_ids=list(range(N_CORES)), trace=trace
    )
    out = np.concatenate(
        [
            res.results[r]["out"]
            .astype(np.float32)
            .transpose(1, 0, 2)
            .reshape(BS, OUT_DIM)
            for r in range(N_CORES)
        ],
        axis=0,
    )
    return out, res


def kernel(x, comp_weight, weight, bias):
    out, _ = _run(x, comp_weight, weight, bias)
    return out
